# revision 55
# baseline (speedup 1.0000x reference)
"""Trainium2 Bass kernel for nn_NeuralEncoder (sparse banded attention encoder).

Sharding: 8 cores = (batch b in 0..3) x (sequence half h in 0..1), uniform SPMD
program over a 1024-row local window per core: h=0 cores get 512 zero-pad rows +
rows 0..511, h=1 cores get rows 0..1023. Each layer shrinks the active window by
128 rows at the front (the CB=128 sliding-window halo); every core emits local
rows 512..1023 as its 512 output rows.

Host<->device transfer is the bottleneck (axon tunnel ~65MB/s, serialized;
multi-stream is slower), so all inputs ship as ONE bf16 wire tensor to core 0
only; cores 1-7 receive device-created zeros (cached across calls — they carry
no data). On device an AllReduce(add) broadcasts the shared weight blob, a
ReduceScatter(add) hands each core its private half-window (own spikes/rope
rows), and a pair-wise AllGather reconstructs the 512-row halo from the (h=0,
h=1) partner, zeroed on h=0 cores via a shipped flag. The band mask is a
shared blob constant; per-key validity rides the softmax Exp's per-partition
bias column. The output is AllGathered on-device so the host fetches a single
bf16 shard from core 0.

Numerics: bf16 matmuls with fp32 PSUM accumulation; LayerNorm, softmax and the
residual stream in fp32. LN gains are folded into the following weight matrices
host-side; the band/padding/spikes_mask is a host-precomputed additive bias
applied to attention scores pre-exp.
"""

import os
import sys

for _p in ("/opt/trn_rl_repo", "/root/.axon_site/_ro/trn_rl_repo"):
    if _p not in sys.path and os.path.isdir(_p):
        sys.path.append(_p)

from concurrent.futures import ThreadPoolExecutor

import numpy as np
import ml_dtypes
import jax
import jax.numpy as jnp
from jax.sharding import Mesh, PartitionSpec, NamedSharding, SingleDeviceSharding
try:
    from jax.experimental.shard_map import shard_map
except ImportError:
    from jax import shard_map

from concourse import bacc
import concourse.tile as tile
from concourse import mybir
from concourse import bass2jax
from concourse.masks import make_identity

# dims
B, T, C, D, H, NH, HD, INTER, L = 4, 1024, 256, 256, 512, 8, 64, 2048, 4
CF, CB, BASE = 0, 128, 10000.0
P = 128
NB = T // P          # 8 local row blocks
N_CORES = 8
NEG = np.float32(-1e30)
F32 = mybir.dt.float32
BF16 = mybir.dt.bfloat16
AF = mybir.ActivationFunctionType

_PROG_CACHE = {}
_EXEC_CACHE = {}


# ---------------------------------------------------------------------------
# wire layout (bf16 elems). Blob = broadcast (shared) region; PC = per-core.
# ---------------------------------------------------------------------------

def _blob_layout(has_bias):
    regions = [("rotm", P * P), ("band", P * 2 * P),
               ("embw", P * 512), ("projw", P * 1024)]
    for l in range(L):
        for nm in ("wq", "wk", "wv", "wo"):
            regions.append((f"{nm}{l}", P * 2048))
        regions.append((f"upw{l}", P * 8192))
        regions.append((f"dnw{l}", P * 8192))
    if has_bias:
        regions.append(("embb", P * 2))
        regions.append(("projb", H))
        for l in range(L):
            regions.append((f"bq{l}", P * 4))
            regions.append((f"bk{l}", P * 4))
            regions.append((f"bv{l}", H))
            regions.append((f"bo{l}", H))
            regions.append((f"upb{l}", P * 16))
            regions.append((f"dnb{l}", H))
    off, out = 0, {}
    for name, n in regions:
        out[name] = (off, n)
        off += n
    return out, off


# per-core region: offsets within each core's PCW-elem chunk. Each core ships
# only its OWN 512 rows of spikes/rope tables; the 512-row halo comes from the
# (h=0, h=1) pair partner via an on-device pair-wise AllGather of the first
# _PC_PAIR elems, zeroed for h=0 cores by the halo flag.
_PC_SPT = 0                      # [128, 2, 512] own spikes.T (local cols 512:1024)
_PC_CST = P * 2 * 512            # [32, 512] own cos (RoPE freqs repeat mod 32)
_PC_SNT = _PC_CST + 32 * 512     # [32, 512] own sin
_PC_PAIR = _PC_SNT + 32 * 512    # pair-AllGathered prefix ends here
_PC_KIV = _PC_PAIR               # [128, 8] additive key-invalid bias (pre-scaled)
_PC_FLG = _PC_KIV + P * NB       # [128, 1] halo flag (h=1 -> 1.0, h=0 -> 0.0)
PCW = _PC_FLG + P


NSPLIT = 4


def _wire_bounds(nw):
    """NSPLIT contiguous chunk bounds covering [0, nw), 1024-aligned."""
    bounds = []
    for i in range(NSPLIT):
        b0 = (nw * i // NSPLIT) // 1024 * 1024
        b1 = (nw * (i + 1) // NSPLIT) // 1024 * 1024 if i < NSPLIT - 1 else nw
        bounds.append((b0, b1))
    return bounds


def _spans(start_block, end_block, max_blocks=4):
    """Split block range [start_block, end_block) into runs of <= max_blocks."""
    out = []
    b = start_block
    while b < end_block:
        e = min(b + max_blocks, end_block)
        out.append((b, e))
        b = e
    return out


def _build_program(has_bias):
    blob_off, blob_elems = _blob_layout(has_bias)
    nw = blob_elems + N_CORES * PCW

    nc = bacc.Bacc("TRN2", target_bir_lowering=False, debug=False,
                   num_devices=N_CORES)

    # the wire splits into NSPLIT inputs, uploaded concurrently from host
    # threads to devices 0..NSPLIT-1 (~6-8% faster than one stream; the GIL
    # otherwise serializes client-side transfer processing). Core i carries
    # chunk i, zeros elsewhere; the AllReduce/ReduceScatter sums of per-core
    # contributions reassemble both regions.
    bounds = _wire_bounds(nw)
    d_wires = [nc.dram_tensor(f"wire{i}", [b1 - b0], BF16, kind="ExternalInput")
               for i, (b0, b1) in enumerate(bounds)]
    d_blob_in = nc.dram_tensor("blob_in", [blob_elems], BF16)
    d_blob = nc.dram_tensor("blob", [blob_elems], BF16, addr_space="Shared")
    d_pc_in = nc.dram_tensor("pc_in", [N_CORES * PCW], BF16)
    d_pc = nc.dram_tensor("pc", [PCW], BF16)
    d_pair = nc.dram_tensor("pair", [2 * _PC_PAIR], BF16)
    d_olocal = nc.dram_tensor("olocal", [T // 2, H], BF16)
    d_og = nc.dram_tensor("og", [N_CORES * (T // 2), H], BF16, addr_space="Shared")
    # four output quarters: host thread-fetches quarter k from device k
    oq = N_CORES * (T // 2) // 4
    d_outq = [nc.dram_tensor(f"outq{k}", [oq, H], BF16, kind="ExternalOutput")
              for k in range(4)]

    def bvw(name, pat, **dims):
        off, n = blob_off[name]
        ap = d_blob.ap()[off:off + n]
        return ap.rearrange(pat, **dims) if pat else ap

    def pcv(off, n, pat, **dims):
        ap = d_pc.ap()[off:off + n]
        return ap.rearrange(pat, **dims) if pat else ap

    with tile.TileContext(nc) as tc:
        with (
            tc.tile_pool(name="consts", bufs=1) as consts,
            tc.tile_pool(name="wts", bufs=2) as wts,
            tc.tile_pool(name="work", bufs=2) as work,
            tc.tile_pool(name="small", bufs=6) as small,
            tc.tile_pool(name="hTs", bufs=2) as hTs,
            tc.tile_pool(name="qk", bufs=1) as qk,
            tc.tile_pool(name="vp", bufs=9) as vp,
            tc.tile_pool(name="es", bufs=3) as es,
            tc.tile_pool(name="itp", bufs=1) as itp,
            tc.tile_pool(name="mm_ps", bufs=3, space="PSUM") as mm_ps,
            tc.tile_pool(name="s_ps", bufs=2, space="PSUM") as s_ps,
            tc.tile_pool(name="o_ps", bufs=2, space="PSUM") as o_ps,
            tc.tile_pool(name="t_ps", bufs=1, space="PSUM") as t_ps,
        ):
            # ---- distribute the wire: broadcast blob, scatter per-core ----
            for i, (b0, b1) in enumerate(bounds):
                lo, hi = max(b0, 0), min(b1, blob_elems)
                if lo < hi:
                    nc.sync.dma_start(out=d_blob_in.ap()[lo:hi],
                                      in_=d_wires[i].ap()[lo - b0:hi - b0])
                lo, hi = max(b0, blob_elems), b1
                if lo < hi:
                    nc.sync.dma_start(
                        out=d_pc_in.ap()[lo - blob_elems:hi - blob_elems],
                        in_=d_wires[i].ap()[lo - b0:hi - b0])
            nc.gpsimd.collective_compute(
                "AllReduce", mybir.AluOpType.add,
                replica_groups=[list(range(N_CORES))],
                ins=[d_blob_in.ap()], outs=[d_blob.ap()])
            nc.gpsimd.collective_compute(
                "ReduceScatter", mybir.AluOpType.add,
                replica_groups=[list(range(N_CORES))],
                ins=[d_pc_in.ap()], outs=[d_pc.ap()])
            # halo exchange within (h=0, h=1) pairs: chunk 0 of d_pair is the
            # even core's block = the batch's global rows [0, 512)
            nc.gpsimd.collective_compute(
                "AllGather", mybir.AluOpType.bypass,
                replica_groups=[[2 * i, 2 * i + 1] for i in range(N_CORES // 2)],
                ins=[d_pc.ap()[0:_PC_PAIR]], outs=[d_pair.ap()])

            # ---- constants ----
            ident = consts.tile([P, P], BF16, tag="ident")
            make_identity(nc, ident[:])
            eps = consts.tile([P, 1], F32, tag="eps")
            nc.vector.memset(eps[:], 1e-5)
            def prv(off, n, pat, **dims):
                ap = d_pair.ap()[off:off + n]
                return ap.rearrange(pat, **dims) if pat else ap

            TH = T // 2
            csT = consts.tile([P, T], BF16, tag="csT")
            snT = consts.tile([P, T], BF16, tag="snT")
            for q in range(4):
                r0, r1 = q * 32, (q + 1) * 32
                nc.sync.dma_start(out=csT[r0:r1, 0:TH],
                                  in_=prv(_PC_CST, 32 * TH, "(p r) -> p r", p=32))
                nc.sync.dma_start(out=csT[r0:r1, TH:T],
                                  in_=pcv(_PC_CST, 32 * TH, "(p r) -> p r", p=32))
                nc.sync.dma_start(out=snT[r0:r1, 0:TH],
                                  in_=prv(_PC_SNT, 32 * TH, "(p r) -> p r", p=32))
                nc.sync.dma_start(out=snT[r0:r1, TH:T],
                                  in_=pcv(_PC_SNT, 32 * TH, "(p r) -> p r", p=32))
            kiv_b = consts.tile([P, NB], BF16, tag="kiv_b")
            nc.sync.dma_start(out=kiv_b[:],
                              in_=pcv(_PC_KIV, P * NB, "(p k) -> p k", p=P))
            kiv = consts.tile([P, NB], F32, tag="kiv")
            nc.scalar.activation(kiv[:], kiv_b[:], AF.Copy)
            hflag_b = consts.tile([P, 1], BF16, tag="hflag_b")
            nc.sync.dma_start(out=hflag_b[:], in_=pcv(_PC_FLG, P, "(p a) -> p a", p=P))
            hflag = consts.tile([P, 1], F32, tag="hflag")
            nc.scalar.activation(hflag[:], hflag_b[:], AF.Copy)
            band = consts.tile([P, 2 * P], BF16, tag="band")
            nc.sync.dma_start(out=band[:], in_=bvw("band", "(p q) -> p q", p=P))
            spT = consts.tile([P, C // P, T], BF16, tag="spT")
            nc.sync.dma_start(out=spT[:, :, TH:T],
                              in_=pcv(_PC_SPT, P * 2 * TH,
                                      "(p c r) -> p c r", p=P, c=C // P))
            nc.sync.dma_start(out=spT[:, :, 0:TH],
                              in_=prv(_PC_SPT, P * 2 * TH,
                                      "(p c r) -> p c r", p=P, c=C // P))
            # zero the halo on h=0 cores (their pair-chunk 0 is their own data)
            nc.vector.tensor_scalar(spT[:, :, 0:TH], spT[:, :, 0:TH],
                                    hflag[:], None, mybir.AluOpType.mult)
            rotm = consts.tile([P, P], BF16, tag="rotm")
            nc.sync.dma_start(out=rotm[:], in_=bvw("rotm", "(p m) -> p m", p=P))
            embw = consts.tile([P, C // P, D], BF16, tag="embw")
            nc.sync.dma_start(out=embw[:],
                              in_=bvw("embw", "(p c d) -> p c d", p=P, c=C // P))
            projw = consts.tile([P, D // P, H], BF16, tag="projw")
            nc.sync.dma_start(out=projw[:],
                              in_=bvw("projw", "(p c h) -> p c h", p=P, c=D // P))
            if has_bias:
                embb_b = consts.tile([P, D // P], BF16, tag="embb_b")
                nc.sync.dma_start(out=embb_b[:],
                                  in_=bvw("embb", "(p c) -> p c", p=P))
                embb = consts.tile([P, D // P], F32, tag="embb")
                nc.scalar.activation(embb[:], embb_b[:], AF.Copy)
                projb = consts.tile([1, H], BF16, tag="projb")
                nc.sync.dma_start(out=projb[:], in_=bvw("projb", "(a h) -> a h", a=1))
                ones_r = consts.tile([1, P], BF16, tag="ones_r")
                nc.vector.memset(ones_r[:], 1.0)

            x = consts.tile([P, NB, H], F32, tag="x")
            gT = consts.tile([P, D // P, T], BF16, tag="gT")

            def mm_group(ps, pairs, bias_row=None):
                """Accumulate lhsT.T @ rhs pairs into ps; optional bias row
                (psum += ones^T @ bias_row) closes the group."""
                for i, (a, bb) in enumerate(pairs):
                    last = (i == len(pairs) - 1) and bias_row is None
                    nc.tensor.matmul(ps, a, bb, start=(i == 0), stop=last)
                if bias_row is not None:
                    nc.tensor.matmul(ps, ones_r[:], bias_row,
                                     start=False, stop=True)

            # ---- embedding: gT = gelu(spikes @ embed_w)^T, x = gT^T @ proj_w ----
            for oc in range(D // P):
                for (s0, s1) in _spans(0, NB):
                    n = (s1 - s0) * P
                    ps = mm_ps.tile([P, 512], F32, tag="mm", name="mmps")[:, :n]
                    for fc in range(C // P):
                        nc.tensor.matmul(ps, embw[:, fc, oc * P:(oc + 1) * P],
                                         spT[:, fc, s0 * P:s0 * P + n],
                                         start=(fc == 0), stop=(fc == C // P - 1))
                    bias = embb[:, oc:oc + 1] if has_bias else 0.0
                    nc.scalar.activation(gT[:, oc, s0 * P:s0 * P + n], ps, AF.Gelu,
                                         bias=bias)
            for rb in range(NB):
                ps = mm_ps.tile([P, 512], F32, tag="mm")
                mm_group(ps,
                         [(gT[:, fc, rb * P:(rb + 1) * P], projw[:, fc, :])
                          for fc in range(D // P)],
                         bias_row=projb[:] if has_bias else None)
                nc.scalar.activation(x[:, rb, :], ps, AF.Copy)

            # ---- layers ----
            _trunc = os.environ.get("KTRUNC", "")
            n_layers = L
            if _trunc.startswith("L"):
                n_layers = int(_trunc[1:].split(":")[0])
            _phase = _trunc.split(":")[1] if ":" in _trunc else "all"
            for l in range(n_layers):
                kb0, qb0 = l, l + 1

                wq = wts.tile([P, H // P, H], BF16, tag="wq")
                nc.sync.dma_start(out=wq[:],
                                  in_=bvw(f"wq{l}", "(p f o) -> p f o", p=P, f=H // P))
                wk = wts.tile([P, H // P, H], BF16, tag="wk")
                nc.sync.dma_start(out=wk[:],
                                  in_=bvw(f"wk{l}", "(p f o) -> p f o", p=P, f=H // P))
                wv = wts.tile([P, H // P, H], BF16, tag="wv")
                nc.sync.dma_start(out=wv[:],
                                  in_=bvw(f"wv{l}", "(p f o) -> p f o", p=P, f=H // P))
                wo = wts.tile([P, H // P, H], BF16, tag="wo")
                nc.sync.dma_start(out=wo[:],
                                  in_=bvw(f"wo{l}", "(p f o) -> p f o", p=P, f=H // P))
                if has_bias:
                    bq_b = wts.tile([P, H // P], BF16, tag="bq_b")
                    nc.sync.dma_start(out=bq_b[:],
                                      in_=bvw(f"bq{l}", "(p c) -> p c", p=P))
                    bq = wts.tile([P, H // P], F32, tag="bq")
                    nc.scalar.activation(bq[:], bq_b[:], AF.Copy)
                    bk_b = wts.tile([P, H // P], BF16, tag="bk_b")
                    nc.sync.dma_start(out=bk_b[:],
                                      in_=bvw(f"bk{l}", "(p c) -> p c", p=P))
                    bk = wts.tile([P, H // P], F32, tag="bk")
                    nc.scalar.activation(bk[:], bk_b[:], AF.Copy)
                    bv = wts.tile([1, H], BF16, tag="bv")
                    nc.sync.dma_start(out=bv[:], in_=bvw(f"bv{l}", "(a h) -> a h", a=1))
                    bo = wts.tile([1, H], BF16, tag="bo")
                    nc.sync.dma_start(out=bo[:], in_=bvw(f"bo{l}", "(a h) -> a h", a=1))
                    dnb = wts.tile([1, H], BF16, tag="dnb")
                    nc.sync.dma_start(out=dnb[:],
                                      in_=bvw(f"dnb{l}", "(a h) -> a h", a=1))
                    upb_b = wts.tile([P, INTER // P], BF16, tag="upb_b")
                    nc.sync.dma_start(out=upb_b[:],
                                      in_=bvw(f"upb{l}", "(p c) -> p c", p=P))
                    upb = wts.tile([P, INTER // P], F32, tag="upb")
                    nc.scalar.activation(upb[:], upb_b[:], AF.Copy)

                def layernorm(src_ap, dst_bf16_ap):
                    stats = small.tile([P, 6], F32, tag="stats")
                    nc.vector.bn_stats(stats[:], src_ap)
                    mv = small.tile([P, 2], F32, tag="mv")
                    nc.vector.bn_aggr(mv[:], stats[:])
                    rstd = small.tile([P, 1], F32, tag="rstd")
                    nc.scalar.activation(rstd[:], mv[:, 1:2], AF.Sqrt, bias=eps[:])
                    nc.vector.reciprocal(rstd[:], rstd[:])
                    nc.vector.tensor_scalar(dst_bf16_ap, src_ap,
                                            mv[:, 0:1], rstd[:],
                                            mybir.AluOpType.subtract,
                                            mybir.AluOpType.mult)

                def transpose128(src_bf16_ap, dst_bf16_ap):
                    # src [128, 128] -> dst [128, 128] via PE transpose
                    tp = t_ps.tile([P, P], BF16, tag="tp")
                    nc.tensor.transpose(tp[:], src_bf16_ap, ident[:])
                    nc.scalar.activation(dst_bf16_ap, tp[:], AF.Copy)

                # LN1 + h^T + v for key range
                hT = hTs.tile([P, H // P, T], BF16, tag="hT")
                vtiles = {}
                for kb in range(kb0, NB):
                    hrow = work.tile([P, H], BF16, tag="hrow")
                    layernorm(x[:, kb, :], hrow[:])
                    for fc in range(H // P):
                        transpose128(hrow[:, fc * P:(fc + 1) * P],
                                     hT[:, fc, kb * P:(kb + 1) * P])
                    ps = mm_ps.tile([P, 512], F32, tag="mm")
                    mm_group(ps,
                             [(hT[:, fc, kb * P:(kb + 1) * P], wv[:, fc, :])
                              for fc in range(H // P)],
                             bias_row=bv[:] if has_bias else None)
                    vt = vp.tile([P, NH, HD + 1], BF16, tag="v")
                    nc.scalar.activation(vt[:, :, 0:HD],
                                         ps.rearrange("p (h d) -> p h d", h=NH),
                                         AF.Copy)
                    nc.vector.memset(vt[:, :, HD:HD + 1], 1.0)
                    vtiles[kb] = vt

                if _phase == "v" and l == n_layers - 1:
                    continue
                # q^T / k^T with RoPE
                qT = qk.tile([P, H // P, T], BF16, tag="qT")
                kT = qk.tile([P, H // P, T], BF16, tag="kT")
                for (dst, w, bias_t, blk0) in (
                    (qT, wq, "bq", qb0),
                    (kT, wk, "bk", kb0),
                ):
                    for oc in range(H // P):
                        for (s0, s1) in _spans(blk0, NB):
                            n = (s1 - s0) * P
                            c0 = s0 * P
                            ps = mm_ps.tile([P, 512], F32, tag="mm", name="mmps")[:, :n]
                            for fc in range(H // P):
                                nc.tensor.matmul(ps, w[:, fc, oc * P:(oc + 1) * P],
                                                 hT[:, fc, c0:c0 + n],
                                                 start=(fc == 0),
                                                 stop=(fc == H // P - 1))
                            q0 = work.tile([P, 512], BF16, tag="q0", name="q0t")[:, :n]
                            if has_bias:
                                bt = bq if bias_t == "bq" else bk
                                nc.scalar.activation(q0, ps, AF.Copy,
                                                     bias=bt[:, oc:oc + 1])
                            else:
                                nc.scalar.activation(q0, ps, AF.Copy)
                            # rope: out = q0 * cs + rot_half(q0) * sn,
                            # rot_half via signed-permutation matmul on PE
                            rp = mm_ps.tile([P, 512], F32, tag="mm", name="rpps")[:, :n]
                            nc.tensor.matmul(rp, rotm[:], q0, start=True, stop=True)
                            t1 = work.tile([P, 512], BF16, tag="t1", name="t1t")[:, :n]
                            nc.vector.tensor_mul(t1, rp, snT[:, c0:c0 + n])
                            t2 = work.tile([P, 512], BF16, tag="t2", name="t2t")[:, :n]
                            nc.vector.tensor_mul(t2, q0, csT[:, c0:c0 + n])
                            nc.vector.tensor_add(dst[:, oc, c0:c0 + n], t1, t2)

                if _phase == "qk" and l == n_layers - 1:
                    continue
                # scores + exp per (kb), then PV/Wo for qb == kb
                estiles = {}
                for kb in range(kb0, NB):
                    qlo, qhi = max(kb, qb0), min(kb + 2, NB)
                    n = (qhi - qlo) * P
                    c0 = qlo * P
                    moff = (qlo - kb) * P
                    for h in range(NH):
                        hp0 = 64 * (h % 2)
                        hc = h // 2
                        sp = s_ps.tile([P, 2 * P], F32, tag="s", name="spt")[:, :n]
                        nc.tensor.matmul(sp,
                                         kT[hp0:hp0 + 64, hc, kb * P:(kb + 1) * P],
                                         qT[hp0:hp0 + 64, hc, c0:c0 + n],
                                         start=True, stop=True)
                        nc.vector.tensor_add(sp, sp, band[:, moff:moff + n])
                        est = es.tile([P, 2 * P], BF16, tag=f"es{h}")
                        nc.scalar.activation(est[:, moff:moff + n], sp, AF.Exp,
                                             scale=0.125, bias=kiv[:, kb:kb + 1])
                        estiles[(h, kb)] = est

                    if kb < qb0 or _phase == "scores":
                        continue
                    qb = kb
                    # PV with appended-ones denominator column
                    ops_ = [o_ps.tile([P, 4, HD + 1], F32, tag="o", name=f"opst{_g}") for _g in range(2)]
                    for h in range(NH):
                        sl = ops_[h // 4][:, h % 4, :]
                        nc.tensor.matmul(sl, estiles[(h, qb)][:, 0:P],
                                         vtiles[qb][:, h, :], start=True, stop=False)
                        nc.tensor.matmul(sl, estiles[(h, qb - 1)][:, P:2 * P],
                                         vtiles[qb - 1][:, h, :], start=False, stop=True)
                    if _phase == "pv1":
                        continue
                    den = small.tile([P, NH], F32, tag="den")
                    nc.scalar.activation(den[:, 0:4], ops_[0][:, :, HD], AF.Copy,
                                         bias=1e-20)
                    nc.scalar.activation(den[:, 4:8], ops_[1][:, :, HD], AF.Copy,
                                         bias=1e-20)
                    nc.vector.reciprocal(den[:], den[:])
                    if _phase == "pv2":
                        continue
                    osc = work.tile([P, H], BF16, tag="osc")
                    for g in range(2):
                        nc.vector.tensor_mul(
                            osc.rearrange("p (g2 h d) -> p g2 h d", g2=2, h=4)[:, g],
                            ops_[g][:, :, 0:HD],
                            den[:, g * 4:(g + 1) * 4, None].to_broadcast((P, 4, HD)))
                    if _phase == "pv":
                        continue
                    oT = work.tile([P, H // P, P], BF16, tag="oT")
                    for fc in range(H // P):
                        transpose128(osc[:, fc * P:(fc + 1) * P], oT[:, fc, :])
                    ps = mm_ps.tile([P, 512], F32, tag="mm")
                    mm_group(ps,
                             [(oT[:, fc, :], wo[:, fc, :]) for fc in range(H // P)],
                             bias_row=bo[:] if has_bias else None)
                    nc.vector.tensor_add(x[:, qb, :], ps, x[:, qb, :])

                if _phase == "attn" and l == n_layers - 1:
                    continue
                # ---- MLP ----
                h2T = hTs.tile([P, H // P, T], BF16, tag="hT")
                for qb in range(qb0, NB):
                    hrow = work.tile([P, H], BF16, tag="hrow")
                    layernorm(x[:, qb, :], hrow[:])
                    for fc in range(H // P):
                        transpose128(hrow[:, fc * P:(fc + 1) * P],
                                     h2T[:, fc, qb * P:(qb + 1) * P])

                for (s0, s1) in _spans(qb0, NB):
                    n = (s1 - s0) * P
                    c0 = s0 * P
                    it = itp.tile([P, INTER // P, 512], BF16, tag="iT")
                    for icg in range(2):
                        uw = wts.tile([P, H // P, INTER // 2], BF16, tag="upw")
                        nc.sync.dma_start(
                            out=uw[:],
                            in_=bvw(f"upw{l}", "(p f i) -> p f i", p=P, f=H // P)[
                                :, :, icg * (INTER // 2):(icg + 1) * (INTER // 2)])
                        for ic in range(INTER // 2 // P):
                            icx = icg * (INTER // 2 // P) + ic
                            ps = mm_ps.tile([P, 512], F32, tag="mm", name="mmps")[:, :n]
                            for fc in range(H // P):
                                nc.tensor.matmul(ps, uw[:, fc, ic * P:(ic + 1) * P],
                                                 h2T[:, fc, c0:c0 + n],
                                                 start=(fc == 0),
                                                 stop=(fc == H // P - 1))
                            bias = upb[:, icx:icx + 1] if has_bias else 0.0
                            nc.scalar.activation(it[:, icx, :n], ps, AF.Gelu,
                                                 bias=bias)
                    dw = [None, None]
                    for icg in range(2):
                        dw[icg] = wts.tile([P, INTER // 2 // P, H], BF16, tag="dnw",
                                           name=f"dnw{icg}")
                        nc.sync.dma_start(
                            out=dw[icg][:],
                            in_=bvw(f"dnw{l}", "(p g o) -> p g o", p=P, g=INTER // P)[
                                :, icg * (INTER // 2 // P):(icg + 1) * (INTER // 2 // P), :])
                    for qb in range(s0, s1):
                        rel = (qb - s0) * P
                        ps = mm_ps.tile([P, 512], F32, tag="mm")
                        mm_group(ps,
                                 [(it[:, icx, rel:rel + P], dw[icx // 8][:, icx % 8, :])
                                  for icx in range(INTER // P)],
                                 bias_row=dnb[:] if has_bias else None)
                        nc.vector.tensor_add(x[:, qb, :], ps, x[:, qb, :])

            # ---- output: local blocks 4..8, bf16, gathered onto every core ----
            xout = consts.tile([P, NB // 2, H], BF16, tag="xout")
            nc.scalar.activation(xout[:], x[:, NB // 2:NB, :], AF.Copy)
            nc.sync.dma_start(
                out=d_olocal.ap().rearrange("(b p) h -> p b h", p=P),
                in_=xout[:])
            nc.gpsimd.collective_compute(
                "AllGather", mybir.AluOpType.bypass,
                replica_groups=[list(range(N_CORES))],
                ins=[d_olocal.ap()], outs=[d_og.ap()])
            for k in range(4):
                nc.sync.dma_start(out=d_outq[k].ap(),
                                  in_=d_og.ap()[k * oq:(k + 1) * oq, :])

    nc.finalize()
    return nc


def _rope_tables():
    inv = 1.0 / (BASE ** (np.arange(0, HD, 2, dtype=np.float32) / np.float32(HD)))
    t = np.arange(T, dtype=np.float32)
    f = t[:, None] * inv[None, :]                      # [T, HD/2]
    emb = np.concatenate([f, f], axis=-1)              # [T, HD]
    return np.cos(emb).astype(np.float32), np.sin(emb).astype(np.float32)


def _bf16(x):
    return np.ascontiguousarray(np.asarray(x, np.float32)).astype(ml_dtypes.bfloat16)


def _pmajor(w, p_groups):
    """[G*128, X] row-major -> [128, G, X] p-major, raveled (bf16)."""
    w = np.asarray(w)
    g = w.shape[0] // P
    return _bf16(w.reshape(g, P, -1).transpose(1, 0, 2)).ravel()


def prepare(inputs):
    """Host-side preprocessing: returns (nc, wire0) — wire0 ships to core 0."""
    inp = {k: np.asarray(v) for k, v in inputs.items()}
    spikes = inp["spikes"].astype(np.float32)          # [B, T, C]
    spikes_mask = inp["spikes_mask"].astype(np.int32)  # [B, T]
    ts = inp["spikes_timestamp"].astype(np.int64)      # [B, T]

    # ---- fold LN gains/biases into weights host-side ----
    ln1_g, ln1_b = inp["ln1_g"].astype(np.float32), inp["ln1_b"].astype(np.float32)
    ln2_g, ln2_b = inp["ln2_g"].astype(np.float32), inp["ln2_b"].astype(np.float32)
    Wq, Wk, Wv, Wo = (inp[k].astype(np.float32) for k in ("Wq", "Wk", "Wv", "Wo"))
    upw, dnw = inp["up_w"].astype(np.float32), inp["down_w"].astype(np.float32)
    bq = inp["bq"].astype(np.float32) + np.einsum("lh,lho->lo", ln1_b, Wq)
    bk = inp["bk"].astype(np.float32) + np.einsum("lh,lho->lo", ln1_b, Wk)
    bv = inp["bv"].astype(np.float32) + np.einsum("lh,lho->lo", ln1_b, Wv)
    bo = inp["bo"].astype(np.float32)
    upb = inp["up_b"].astype(np.float32) + np.einsum("lh,lhi->li", ln2_b, upw)
    dnb = inp["down_b"].astype(np.float32)
    wq_eff = ln1_g[:, :, None] * Wq
    wk_eff = ln1_g[:, :, None] * Wk
    wv_eff = ln1_g[:, :, None] * Wv
    upw_eff = ln2_g[:, :, None] * upw

    has_bias = bool(
        np.abs(inp["embed_b"]).max() > 0 or np.abs(inp["proj_b"]).max() > 0
        or max(np.abs(a).max() for a in (bq, bk, bv, bo, upb, dnb)) > 0)

    key = has_bias
    if key not in _PROG_CACHE:
        _PROG_CACHE[key] = _build_program(has_bias)
    nc = _PROG_CACHE[key]

    blob_off, blob_elems = _blob_layout(has_bias)

    # signed permutation for rotate-half: out[m] = sign(m) * q[partner(m)]
    # (as matmul rotm.T @ q: rotm[partner(m), m] = sign(m))
    rotm_np = np.zeros((P, P), np.float32)
    for m in range(P):
        d = m % HD
        partner = m + HD // 2 if d < HD // 2 else m - HD // 2
        rotm_np[partner, m] = -1.0 if d < HD // 2 else 1.0

    blob = np.zeros(blob_elems, ml_dtypes.bfloat16)

    def put(name, arr_flat):
        off, n = blob_off[name]
        assert arr_flat.size == n, (name, arr_flat.size, n)
        blob[off:off + n] = arr_flat

    put("rotm", _bf16(rotm_np).ravel())
    # band structure in local coords, shared by all cores and key blocks:
    # col block 0 (q in same block as k): allow kc <= qc; col block 1
    # (q one block above k): allow kc >= qc.
    kc_ = np.arange(P)[:, None]
    qc_ = np.arange(P)[None, :]
    band_np = np.concatenate(
        [np.where(kc_ <= qc_, 0.0, NEG), np.where(kc_ >= qc_, 0.0, NEG)],
        axis=1).astype(np.float32)
    put("band", _bf16(band_np).ravel())
    put("embw", _pmajor(inp["embed_w"], 2))
    put("projw", _pmajor(inp["proj_w"], 2))
    for l in range(L):
        put(f"wq{l}", _pmajor(wq_eff[l], 4))
        put(f"wk{l}", _pmajor(wk_eff[l], 4))
        put(f"wv{l}", _pmajor(wv_eff[l], 4))
        put(f"wo{l}", _pmajor(Wo[l], 4))
        put(f"upw{l}", _pmajor(upw_eff[l], 4))
        put(f"dnw{l}", _pmajor(dnw[l], 16))
    if has_bias:
        put("embb", _bf16(inp["embed_b"].reshape(2, P).T).ravel())
        put("projb", _bf16(inp["proj_b"]).ravel())
        for l in range(L):
            put(f"bq{l}", _bf16(bq[l].reshape(4, P).T).ravel())
            put(f"bk{l}", _bf16(bk[l].reshape(4, P).T).ravel())
            put(f"bv{l}", _bf16(bv[l]).ravel())
            put(f"bo{l}", _bf16(bo[l]).ravel())
            put(f"upb{l}", _bf16(upb[l].reshape(16, P).T).ravel())
            put(f"dnb{l}", _bf16(dnb[l]).ravel())

    cos_t, sin_t = _rope_tables()   # [T, HD]

    pcs = []
    TH = T // 2
    for b in range(B):
        for h in range(2):
            g0 = h * TH             # global row of local row 512
            # own rows: global [g0, g0+512) live at local cols [512, 1024)
            spT_own = spikes[b, g0:g0 + TH, :].T       # [C, 512]

            ts_own = ts[b, g0:g0 + TH]
            cs_own = cos_t[ts_own][:, 0:HD // 2].T     # [32, 512]
            sn_own = sin_t[ts_own][:, 0:HD // 2].T

            # per-key additive invalid bias [kc, kb], pre-scaled by 0.125
            # (rides the Exp activation's per-partition bias column)
            gk = (np.arange(T) - TH + g0)
            kvalid = (gk >= 0) & (spikes_mask[b, np.clip(gk, 0, T - 1)] > 0)
            kiv = np.where(kvalid, 0.0, NEG * 0.125).astype(np.float32)
            kiv = kiv.reshape(NB, P).T                 # [128, NB]

            pc = np.empty(PCW, ml_dtypes.bfloat16)
            pc[_PC_SPT:_PC_SPT + P * 2 * TH] = _pmajor(spT_own, 2)
            pc[_PC_CST:_PC_CST + 32 * TH] = _bf16(cs_own).ravel()
            pc[_PC_SNT:_PC_SNT + 32 * TH] = _bf16(sn_own).ravel()
            pc[_PC_KIV:_PC_KIV + P * NB] = _bf16(kiv).ravel()
            pc[_PC_FLG:_PC_FLG + P] = ml_dtypes.bfloat16(float(h))
            pcs.append(pc)

    wire0 = np.concatenate([blob] + pcs)
    nw = blob_elems + N_CORES * PCW
    return nc, tuple(np.ascontiguousarray(wire0[b0:b1])
                     for b0, b1 in _wire_bounds(nw))


# ---------------------------------------------------------------------------
# cached-jit runner: wire ships to core 0 only; cores 1..7 get device zeros
# ---------------------------------------------------------------------------

def _get_exec(nc):
    key = id(nc)
    if key in _EXEC_CACHE:
        return _EXEC_CACHE[key]
    bass2jax.install_neuronx_cc_hook()
    partition_name = nc.partition_id_tensor.name if nc.partition_id_tensor else None
    in_names, out_names, out_avals, zero_shapes = [], [], [], []
    for alloc in nc.m.functions[0].allocations:
        if not isinstance(alloc, mybir.MemoryLocationSet):
            continue
        name = alloc.memorylocations[0].name
        if alloc.kind == "ExternalInput":
            if name != partition_name:
                in_names.append(name)
        elif alloc.kind == "ExternalOutput":
            shape = tuple(alloc.tensor_shape)
            dtype = mybir.dt.np(alloc.dtype)
            out_names.append(name)
            out_avals.append(jax.core.ShapedArray(shape, dtype))
            zero_shapes.append((shape, dtype))
    assert nc.dbg_addr is None, "runner assumes debug=False"
    assert in_names == [f"wire{i}" for i in range(NSPLIT)], in_names
    assert out_names == [f"outq{k}" for k in range(4)], out_names
    n_params = len(in_names)
    n_outs = len(out_avals)
    all_names = list(in_names) + list(out_names)
    if partition_name is not None:
        all_names.append(partition_name)
    donate = tuple(range(n_params, n_params + n_outs))

    def _body(*args):
        operands = list(args)
        if partition_name is not None:
            operands.append(bass2jax.partition_id_tensor())
        outs = bass2jax._bass_exec_p.bind(
            *operands,
            out_avals=tuple(out_avals),
            in_names=tuple(all_names),
            out_names=tuple(out_names),
            lowering_input_output_aliases=(),
            sim_require_finite=True,
            sim_require_nnan=True,
            nc=nc,
        )
        return tuple(outs)

    devices = jax.devices()[:N_CORES]
    mesh = Mesh(np.asarray(devices), ("core",))
    in_specs = (PartitionSpec("core"),) * (n_params + n_outs)
    out_specs = (PartitionSpec("core"),) * n_outs
    sharded = jax.jit(
        shard_map(_body, mesh=mesh, in_specs=in_specs, out_specs=out_specs,
                  check_rep=False),
        donate_argnums=donate, keep_unused=True)

    core_sharding = NamedSharding(mesh, PartitionSpec("core"))
    zeros_out = jax.jit(
        lambda: tuple(jnp.zeros((N_CORES * s[0], *s[1:]), d)
                      for s, d in zero_shapes),
        out_shardings=(core_sharding,) * n_outs)

    st = dict(sharded=sharded, devices=devices, core_sharding=core_sharding,
              zeros_out=zeros_out, out_names=out_names, zero_dev=None)
    _EXEC_CACHE[key] = st
    return st


def run_model(nc, wires):
    """One full inference: ship NSPLIT wire chunks to cores 0..NSPLIT-1 from
    host threads, run, thread-fetch the two output halves [B, T, H]."""
    st = _get_exec(nc)
    devices = st["devices"]
    if st["zero_dev"] is None:
        # persistent zero padding shards (inputs, not donated): chunk i is
        # real on core i, zeros everywhere else
        def zmake(shape, dtype, d):
            return jax.jit(lambda: jnp.zeros(shape, dtype),
                           out_shardings=SingleDeviceSharding(d))()
        st["zero_dev"] = [
            [zmake(w.shape, w.dtype, d) for d in devices]
            for w in wires]
        st["pool"] = ThreadPoolExecutor(NSPLIT)
    futs = [st["pool"].submit(jax.device_put, w, devices[i])
            for i, w in enumerate(wires)]
    wire_gs = []
    for i, (w, f) in enumerate(zip(wires, futs)):
        shards = list(st["zero_dev"][i])
        shards[i] = f.result()
        wire_gs.append(jax.make_array_from_single_device_arrays(
            (N_CORES * w.shape[0],), st["core_sharding"], shards))
    zouts = st["zeros_out"]()
    out_arrs = st["sharded"](*wire_gs, *zouts)
    # outputs were AllGathered on-device: every core holds the full result;
    # thread-fetch quarter k from device k, each thread converting its
    # cores' slices straight into the f32 output
    shards = [out_arrs[k].addressable_shards[k].data for k in range(4)]
    for s in shards:
        s.copy_to_host_async()
    out = np.empty((B, T, H), np.float32)

    def land(shard_arr, c0):
        res = np.asarray(shard_arr).reshape(-1, T // 2, H)
        for i in range(res.shape[0]):
            b, h = divmod(c0 + i, 2)
            out[b, h * (T // 2):(h + 1) * (T // 2), :] = res[i]

    futs = [st["pool"].submit(land, s, k * (N_CORES // 4))
            for k, s in enumerate(shards)]
    for f in futs:
        f.result()
    return out


def kernel(**inputs):
    nc, wires = prepare(inputs)
    return run_model(nc, wires)
